# revision 1
# baseline (speedup 1.0000x reference)
"""HiRA layer (rank-modulated linear) Trainium2 kernel.

Computes out = x @ (W * (1 + A^T B^T)^T)^T + bias for
x:[4,2048,4096] f32, W:[4096,4096], A:[16,4096], B:[4096,16], bias:[4096].

Sharding: 2-way over tokens x 4-way over out-features (8 NeuronCores).
Each core:
  1. builds its adapted-weight shard on device:
     P'[i,o] = sum_r A_aug[r,i] * B_aug^T[r,o]   (ones-row augmentation
     folds the +1 into the matmul), then AWT[i,o] = W^T[i,o] * P'[i,o]
     cast to bf16, resident in SBUF.
  2. streams x tiles (host pre-blocked to [m, p=i, k, t=tok] bf16) through
     the PE: psum[tok,o] = sum_k XB[m,:,k,:].T @ AWT[k-chunk, o-slice],
     adds bias on DVE during the PSUM->SBUF copy, DMAs out f32.

Host side only reshapes/transposes/casts and slices shards; every FLOP of
the reference computation happens on device.
"""

import sys

for _p in ("/opt/trn_rl_repo",):
    if _p not in sys.path:
        sys.path.insert(0, _p)

import numpy as np
import ml_dtypes

BF16 = ml_dtypes.bfloat16

# problem shape (hardcoded per contract)
B, S, IN, OUT, R = 4, 2048, 4096, 4096, 16
TOK = B * S            # 8192
TB, OB = 2, 4          # token-halves x out-feature quarters = 8 cores
TOKH = TOK // TB       # 4096 tokens per core
OQ = OUT // OB         # 1024 out features per core
MT = TOKH // 128       # 32 token tiles
KT = IN // 128         # 32 contraction chunks
NG = OQ // 512         # 2 psum groups of 512 outputs
N_CORES = 8

TRACE = False          # test.py sets True to capture NTFF exec time
LAST_RESULT = None     # BassKernelResults of the most recent run

_NC_CACHE = None


def _build_nc():
    import concourse.bass as bass
    import concourse.bacc as bacc
    import concourse.mybir as mybir
    from concourse import tile

    f32 = mybir.dt.float32
    bf16 = mybir.dt.bfloat16

    nc = bacc.Bacc(
        "TRN2", target_bir_lowering=False, debug=False, num_devices=N_CORES
    )

    XB = nc.dram_tensor("xb", [MT, 128, KT, 128], bf16, kind="ExternalInput")
    WT = nc.dram_tensor("wt", [KT, 128, OQ], f32, kind="ExternalInput")
    AAUG = nc.dram_tensor("a_aug", [R + 1, IN], bf16, kind="ExternalInput")
    BTAUG = nc.dram_tensor("bt_aug", [R + 1, OQ], bf16, kind="ExternalInput")
    BIASB = nc.dram_tensor("bias_b", [128, OQ], f32, kind="ExternalInput")
    OUTP = nc.dram_tensor("out", [MT, 128, OQ], f32, kind="ExternalOutput")

    with tile.TileContext(nc) as tc:
        with (
            tc.tile_pool(name="const", bufs=1) as const,
            tc.tile_pool(name="awt", bufs=1) as awtp,
            tc.tile_pool(name="wtld", bufs=6) as wtld,
            tc.tile_pool(name="xb", bufs=5) as xbp,
            tc.tile_pool(name="ob", bufs=4) as obp,
            tc.tile_pool(name="ppsum", bufs=4, space=bass.MemorySpace.PSUM) as ppp,
            tc.tile_pool(name="opsum", bufs=3, space=bass.MemorySpace.PSUM) as opp,
            tc.tile_pool(name="wupsum", bufs=1, space=bass.MemorySpace.PSUM) as wup,
        ):
            a_t = const.tile([R + 1, IN], bf16)
            bt_t = const.tile([R + 1, OQ], bf16)
            bias_t = const.tile([128, OQ], f32)
            nc.sync.dma_start(out=a_t[:], in_=AAUG[:])
            nc.sync.dma_start(out=bt_t[:], in_=BTAUG[:])
            nc.sync.dma_start(out=bias_t[:], in_=BIASB[:])

            # adapted weight, bf16, resident: [p=i%128, k=i//128, o]
            awt = awtp.tile([128, KT, OQ], bf16)

            # PE warmup burst: ~12 back-to-back matmuls on scratch data keep
            # the PE busy through one full HAM SHORT window so the clock
            # un-gates (1.2 -> 2.4 GHz) before the real matmul stream starts.
            wu_l = const.tile([128, 128], bf16)
            wu_r = const.tile([128, 512], bf16)
            nc.vector.memset(wu_l[:], 0.0)
            nc.vector.memset(wu_r[:], 0.0)
            wu_p = wup.tile([128, 512], f32)

            def scratch_mm(n=1):
                for _ in range(n):
                    nc.tensor.matmul(
                        wu_p[:], wu_l[:], wu_r[:], start=True, stop=True
                    )

            scratch_mm(12)

            def mod_chunk(og, k):
                """AWT[:, k, og] = (A_aug^T @ B_aug^T) * W^T for one chunk."""
                osl = slice(og * 512, (og + 1) * 512)
                wt_t = wtld.tile([128, 512], f32, tag="wt")
                nc.sync.dma_start(out=wt_t[:], in_=WT[k, :, osl])
                pp_t = ppp.tile([128, 512], f32, tag="pp")
                nc.tensor.matmul(
                    pp_t[:],
                    a_t[:, k * 128:(k + 1) * 128],
                    bt_t[:, osl],
                    start=True,
                    stop=True,
                )
                nc.vector.tensor_mul(awt[:, k, osl], pp_t[:], wt_t[:])

            def main_tile(m, og, xb_t):
                """One [128 tok, 512 out] output tile: accumulate over k."""
                osl = slice(og * 512, (og + 1) * 512)
                po_t = opp.tile([128, 512], f32, tag="po")
                for k in range(KT):
                    nc.tensor.matmul(
                        po_t[:],
                        xb_t[:, k, :],
                        awt[:, k, osl],
                        start=(k == 0),
                        stop=(k == KT - 1),
                    )
                o_t = obp.tile([128, 512], f32, tag="ot")
                nc.vector.tensor_add(o_t[:], po_t[:], bias_t[:, osl])
                nc.sync.dma_start(out=OUTP[m, :, osl], in_=o_t[:])

            # Two passes over og so the PE can start og0 main matmuls while
            # og1 modulation chunks are still being produced by the DVE.
            # Pass 1 (og0): m=0's accumulation is interleaved with the og0
            # modulation chunks (both proceed at DVE chunk-production pace,
            # keeping the PE warm instead of idling until all chunks exist).
            xb0 = xbp.tile([128, KT, 128], bf16, tag="xb")
            nc.sync.dma_start(out=xb0[:], in_=XB[0])
            po0 = opp.tile([128, 512], f32, tag="po")
            scratch_mm(8)  # absorb first-chunk DMA latency, stay warm
            for k in range(KT):
                mod_chunk(0, k)
                scratch_mm(1)  # keep PE saturated while DVE makes chunk k
                nc.tensor.matmul(
                    po0[:],
                    xb0[:, k, :],
                    awt[:, k, 0:512],
                    start=(k == 0),
                    stop=(k == KT - 1),
                )
            o_t = obp.tile([128, 512], f32, tag="ot")
            nc.vector.tensor_add(o_t[:], po0[:], bias_t[:, 0:512])
            nc.sync.dma_start(out=OUTP[0, :, 0:512], in_=o_t[:])

            # Pass 1 continued: m=1..31 for og0; og1 modulation chunks are
            # drip-fed two per m-iteration (done by m=16).
            for m in range(1, MT):
                for j in (2 * (m - 1), 2 * m - 1):
                    if j < KT:
                        mod_chunk(1, j)
                xb_t = xbp.tile([128, KT, 128], bf16, tag="xb")
                nc.sync.dma_start(out=xb_t[:], in_=XB[m])
                main_tile(m, 0, xb_t)

            # Pass 2 (og1): all chunks ready long before these run.
            for m in range(MT):
                xb_t = xbp.tile([128, KT, 128], bf16, tag="xb")
                nc.sync.dma_start(out=xb_t[:], in_=XB[m])
                main_tile(m, 1, xb_t)

    nc.compile()
    return nc


def _get_nc():
    global _NC_CACHE
    if _NC_CACHE is None:
        _NC_CACHE = _build_nc()
    return _NC_CACHE


def kernel(x, weight, bias, lora_A, lora_B):
    global LAST_RESULT
    from concourse.bass_utils import run_bass_kernel_spmd

    x = np.asarray(x, dtype=np.float32)
    weight = np.asarray(weight, dtype=np.float32)
    bias = np.asarray(bias, dtype=np.float32)
    lora_A = np.asarray(lora_A, dtype=np.float32)
    lora_B = np.asarray(lora_B, dtype=np.float32)

    x2 = x.reshape(TOK, IN)

    # x blocked per token-half: [m, p=i%128, k=i//128, t=tok%128] bf16
    xbs = []
    for tb in range(TB):
        xh = x2[tb * TOKH:(tb + 1) * TOKH]
        xb = xh.reshape(MT, 128, KT, 128).transpose(0, 3, 2, 1)  # [m,p,k,t]
        xbs.append(np.ascontiguousarray(xb.astype(BF16)))

    a_aug = np.concatenate(
        [lora_A, np.ones((1, IN), np.float32)], axis=0
    ).astype(BF16)

    wts, bts, biases = [], [], []
    for ob in range(OB):
        osl = slice(ob * OQ, (ob + 1) * OQ)
        wq = weight[osl]                                   # [OQ, IN]
        wts.append(np.ascontiguousarray(wq.T.reshape(KT, 128, OQ)))  # f32
        bq = lora_B[osl]                                   # [OQ, R]
        bts.append(
            np.concatenate(
                [bq.T, np.ones((1, OQ), np.float32)], axis=0
            ).astype(BF16)
        )
        biases.append(np.ascontiguousarray(np.tile(bias[osl][None, :], (128, 1))))

    in_maps = []
    for c in range(N_CORES):
        tb, ob = c // OB, c % OB
        in_maps.append(
            {
                "xb": xbs[tb],
                "wt": wts[ob],
                "a_aug": a_aug,
                "bt_aug": bts[ob],
                "bias_b": biases[ob],
            }
        )

    nc = _get_nc()
    res = run_bass_kernel_spmd(
        nc, in_maps, core_ids=list(range(N_CORES)), trace=TRACE
    )
    LAST_RESULT = res

    # reassemble: out[c] is [MT, 128, OQ] -> [TOKH, OQ]
    halves = []
    for tb in range(TB):
        cols = [
            res.results[tb * OB + ob]["out"].reshape(TOKH, OQ)
            for ob in range(OB)
        ]
        halves.append(np.concatenate(cols, axis=1))
    full = np.concatenate(halves, axis=0).reshape(B, S, OUT)
    return full

